# revision 38
# baseline (speedup 1.0000x reference)
"""MHA kernel for Trainium2, 8 NeuronCores.

Sharding: (batch, query-half) -> 8 shards. Core c handles batch c//2,
query rows (c%2)*1024:(c%2+1)*1024. Each core computes all 16 heads for its
1024 query rows. K/V projections for head-pairs 0-3 are computed locally on
every core (fast start); K/V for head-pairs 4-7 are DEDUPLICATED across the
two cores sharing a batch: core parity p computes pairs {4+p, 6+p}, stages
them to DRAM, and a pairwise AllGather ([[0,1],[2,3],[4,5],[6,7]]) exchanges
them while the early attention runs. Gather round r delivers pairs
(4+2r, 5+2r) in natural consumption order - no weight permutation needed.
Output rows stay disjoint across cores -> no output collectives.

Host-side marshaling (part of sharding): X slices transposed and cast to
bf16 ([D, S] layout); weights pre-packed (per-core wk/wv carry pairs 0-3
plus the local exchange pairs). ALL inputs are concatenated into ONE bf16
blob per core (~30us of per-execution overhead is charged PER bound input
tensor). Output is written bf16 and upcast to f32 on the host.

Per-core compute (bf16 matmuls, f32 PSUM):
  qT per pair [128 = 64+64, SQ] via 8-chunk accumulated matmuls; K/V
  likewise, biases folded into the PSUM->SBUF drains. v natural [s, 130]
  with interleaved ones columns for softmax rowsums. scores^T per pair via
  K=64 matmuls packed per k-chunk into a shared psum tile so both heads
  dispatch back-to-back and run CONCURRENTLY in disjoint PE row groups.
  E = exp on ScalarE; z'^T accumulated with the ones-column trick;
  z = z' * approx-recip(rowsum); out = z^T.T @ Wo + bo.

Scheduling: attention at natural priority; projection work for future
positions at LOW filler priorities (q/k tier above v tier); normalize at
the lowest DVE tier. The Tile scheduler slots fillers into PE-idle moments
while ACT chews exp, keeping the PE dense so the HAM clock gate stays at
8/8 (2.4 GHz).
"""

import os

import numpy as np
import ml_dtypes



import concourse.bass as bass
import concourse.tile as tile
from concourse import bacc, mybir
from concourse.bass import ds, ts
from concourse.bass_utils import run_bass_kernel_spmd

B, S, D = 4, 2048, 1024
H, DK, DV = 16, 64, 64
N_CORES = 8
SQ = S // 2  # query rows per core
P = 128
NHP = H // 2   # head pairs (positions)
NKL = 6        # k pairs computed locally: 0-3 + two exchange pairs
F32 = mybir.dt.float32
BF = mybir.dt.bfloat16
EXP = mybir.ActivationFunctionType.Exp

FILLER = -10_000_000      # q/k proj steps (gate the exp supply)
FILLER_V = -15_000_000    # v proj steps
NORM_PRI = -25_000_000    # normalize chain: lowest

# DRAM exchange layout per round: [P, 2048] bf16 (k only)
KCOLS = S
XCOLS = KCOLS

# blob layout (bf16 elements)
SZ_W = P * 8 * 1024           # wq / wo / wv (v is fully local)
SZ_WK = P * 8 * (NKL * P)     # wk: 6 pairs = 768 cols
SZ_XQ = D * SQ
SZ_XKV = D * S
OFF_WQ = 0
OFF_XQ = OFF_WQ + SZ_W
OFF_WK = OFF_XQ + SZ_XQ
OFF_XK = OFF_WK + SZ_WK
OFF_WV = OFF_XK + SZ_XKV
OFF_XV = OFF_WV + SZ_W
OFF_WO = OFF_XV + SZ_XKV
OFF_BQ = OFF_WO + SZ_W
OFF_BK = OFF_BQ + P * NHP
OFF_BVB = OFF_BK + P * NKL
BLOB_N = OFF_BVB + P * H * DV

GROUPS = [[0, 1], [2, 3], [4, 5], [6, 7]]


def build_kernel(nc, tc, VARIANT=""):
    blob = nc.declare_dram_parameter("blob", [BLOB_N], BF, isOutput=False).ap()
    xqT_d = blob[ds(OFF_XQ, SZ_XQ)].rearrange("(p c) -> p c", p=D)
    xkT_d = blob[ds(OFF_XK, SZ_XKV)].rearrange("(p c) -> p c", p=D)
    xvT_d = blob[ds(OFF_XV, SZ_XKV)].rearrange("(p c) -> p c", p=D)
    wq_d = blob[ds(OFF_WQ, SZ_W)].rearrange("(p c k) -> p c k", p=P, c=8)
    wk_d = blob[ds(OFF_WK, SZ_WK)].rearrange("(p c k) -> p c k", p=P, c=8)
    wv_d = blob[ds(OFF_WV, SZ_W)].rearrange("(p c k) -> p c k", p=P, c=8)
    wo_d = blob[ds(OFF_WO, SZ_W)].rearrange("(t p c k) -> t p c k", t=2, p=P, c=8)
    bq_d = blob[ds(OFF_BQ, P * NHP)].rearrange("(p c) -> p c", p=P)
    bk_d = blob[ds(OFF_BK, P * NKL)].rearrange("(p c) -> p c", p=P)
    bvb_d = blob[ds(OFF_BVB, P * H * DV)].rearrange("(p c) -> p c", p=P)
    out = nc.declare_dram_parameter("out", [SQ, D], BF, isOutput=True).ap()

    import contextlib

    ctx = contextlib.ExitStack()
    with ctx:
        consts = ctx.enter_context(tc.tile_pool(name="consts", bufs=1))
        wpool = ctx.enter_context(tc.tile_pool(name="wpool", bufs=1))
        xtp = ctx.enter_context(tc.tile_pool(name="xtp", bufs=1))
        ztp = ctx.enter_context(tc.tile_pool(name="ztp", bufs=1))
        qpool = ctx.enter_context(tc.tile_pool(name="qpool", bufs=2))
        kpool = ctx.enter_context(tc.tile_pool(name="kpool", bufs=4))
        epool = ctx.enter_context(tc.tile_pool(name="epool", bufs=3))
        rbpool = ctx.enter_context(tc.tile_pool(name="rbpool", bufs=2))
        opool = ctx.enter_context(tc.tile_pool(name="opool", bufs=2))
        wopool = ctx.enter_context(tc.tile_pool(name="wopool", bufs=2))
        vqp = ctx.enter_context(tc.tile_pool(name="vqp", bufs=2))
        drp = ctx.enter_context(tc.tile_pool(name="drp", bufs=1, space="DRAM"))
        pp = ctx.enter_context(tc.tile_pool(name="pp", bufs=2, space=bass.MemorySpace.PSUM))
        sp = ctx.enter_context(tc.tile_pool(name="sp", bufs=2, space=bass.MemorySpace.PSUM))
        zp = ctx.enter_context(tc.tile_pool(name="zp", bufs=2, space=bass.MemorySpace.PSUM))

        RECIP = nc.vector.reciprocal_approx_fast

        din = [drp.tile([P, XCOLS], BF, tag=f"din{i}", name=f"din{i}")
               for i in range(2)]
        dout = [drp.tile([2, P, XCOLS], BF, tag=f"dout{i}", name=f"dout{i}")
                for i in range(2)]

        # ---- constants ----
        bqc_bf = consts.tile([P, NHP], BF, tag="bqc_bf")
        bkc_bf = consts.tile([P, NKL], BF, tag="bkc_bf")
        nc.sync.dma_start(out=bqc_bf[:, :], in_=bq_d[:, :])
        nc.scalar.dma_start(out=bkc_bf[:, :], in_=bk_d[:, :])
        bqc = consts.tile([P, NHP], F32, tag="bqc")
        bkc = consts.tile([P, NKL], F32, tag="bkc")
        nc.vector.tensor_copy(bqc[:, :], bqc_bf[:, :])
        nc.vector.tensor_copy(bkc[:, :], bkc_bf[:, :])
        bvb = consts.tile([P, H * DV], BF, tag="bvb")

        wq_sb = wpool.tile([P, 8, H * DK], BF, tag="wq_sb")
        wk_sb = wpool.tile([P, 8, NKL * P], BF, tag="wk_sb")
        wv_sb = wpool.tile([P, 8, H * DV], BF, tag="wv_sb")

        xqT = xtp.tile([P, 8, SQ], BF, tag="xqT")
        xkT = xtp.tile([P, 8, S], BF, tag="xkT")
        xvT = xtp.tile([P, 8, S], BF, tag="xvT")
        _k = [0]

        def E():
            e = (nc.sync, nc.scalar)[_k[0] % 2]
            _k[0] += 1
            return e

        for dc in range(8):
            E().dma_start(out=wq_sb[:, dc, :], in_=wq_d[:, dc, :])
            E().dma_start(out=xqT[:, dc, :], in_=xqT_d[ds(dc * P, P), :])
        E().dma_start(out=wk_sb[:, :, :], in_=wk_d[:, :, :])
        for dc in range(8):
            E().dma_start(out=xkT[:, dc, :], in_=xkT_d[ds(dc * P, P), :])
        E().dma_start(out=wv_sb[:, :, :], in_=wv_d[:, :, :])
        E().dma_start(out=bvb[:, :], in_=bvb_d[:, :])
        for sq in range(4):
            for dc in range(8):
                E().dma_start(out=xvT[:, dc, ts(sq, 512)],
                              in_=xvT_d[ds(dc * P, P), ts(sq, 512)])
        wo_sb0 = wopool.tile([P, 8, 512], BF, tag="wo_sb")
        E().dma_start(out=wo_sb0[:, :, :], in_=wo_d[0, :, :, :])
        wo_sb1 = wopool.tile([P, 8, 512], BF, tag="wo_sb")
        E().dma_start(out=wo_sb1[:, :, :], in_=wo_d[1, :, :, :])

        zT = ztp.tile([P, NHP, SQ], BF, tag="zT")
        NJ = S // P  # 16 k-chunks

        def outproj_step(qc, dt):
            wo_sb = wo_sb0 if dt == 0 else wo_sb1
            ps = pp.tile([P, 512], F32, tag="proj")
            for fc in range(8):
                nc.tensor.matmul(ps[:, :], zT[:, fc, ts(qc, P)],
                                 wo_sb[:, fc, :],
                                 start=(fc == 0), stop=(fc == 7))
            o_t = opool.tile([P, 512], BF, tag="o")
            nc.vector.tensor_copy(o_t[:, :], ps[:, :])
            nc.sync.dma_start(out=out[ts(qc, P), ts(dt, 512)], in_=o_t[:, :])

        # ---- v projection for a 2-pair group g in {0:(0,1), 1:(2,3),
        #      2:(exchange pairs)} of the local wv packing ----
        def make_vq_steps(g2):
            v_q = vqp.tile([P, S // P, 2, 130], BF, tag="v_q")
            vv = v_q.rearrange("p s h (a c) -> p s h a c", a=2)
            gsl = ds(g2 * 256, 256)
            steps = [lambda: nc.vector.memset(vv[:, :, :, :, 64:65], 1.0)]

            def step(sc):
                def emit():
                    ps = pp.tile([P, 512], F32, tag="proj")
                    for dc in range(8):
                        nc.tensor.matmul(ps[:, 0:256], xvT[:, dc, ts(sc, P)],
                                         wv_sb[:, dc, gsl],
                                         start=(dc == 0), stop=(dc == 7))
                    nc.vector.tensor_add(
                        vv[:, sc, :, :, 0:64],
                        ps[:, 0:256].rearrange("p (h a c) -> p h a c", h=2, a=2),
                        bvb[:, gsl].rearrange("p (h a c) -> p h a c", h=2, a=2),
                    )
                return emit

            steps += [step(sc) for sc in range(S // P)]
            return v_q, steps

        # ---- q projection steps for one position ----
        def make_q_steps(u):
            hsl = ts(u, P)
            q_t = qpool.tile([P, SQ], BF, tag="q_t")
            steps = []
            state = {}

            def half(col, lo):
                def emit():
                    if lo:
                        ps = pp.tile([P, 512], F32, tag="proj")
                        state[col] = ps
                    else:
                        ps = state.pop(col)
                    for dc in range(4):
                        d = dc if lo else dc + 4
                        nc.tensor.matmul(ps[:, :], wq_sb[:, d, hsl],
                                         xqT[:, d, ds(col, 512)],
                                         start=(d == 0), stop=(d == 7))
                    if not lo:
                        nc.vector.tensor_scalar_add(
                            q_t[:, ds(col, 512)], ps[:, :], bqc[:, u:u + 1])
                return emit

            for qt in range(SQ // 512):
                for lo in (True, False):
                    steps.append(half(qt * 512, lo))
            return q_t, steps

        # ---- k projection for local slot l (0-3 = pairs 0-3; 4,5 =
        #      exchange pairs, staged to DRAM round l-4) ----
        def make_k_steps(l):
            hsl = ts(l, P)
            k_t = kpool.tile([P, S], BF, tag="k_t")
            steps = []
            state = {}

            def half(col, lo):
                def emit():
                    if lo:
                        ps = pp.tile([P, 512], F32, tag="proj")
                        state[col] = ps
                    else:
                        ps = state.pop(col)
                    for dc in range(4):
                        d = dc if lo else dc + 4
                        nc.tensor.matmul(ps[:, :], wk_sb[:, d, hsl],
                                         xkT[:, d, ds(col, 512)],
                                         start=(d == 0), stop=(d == 7))
                    if not lo:
                        nc.vector.tensor_scalar_add(
                            k_t[:, ds(col, 512)], ps[:, :], bkc[:, l:l + 1])
                return emit

            for st in range(S // 512):
                for lo in (True, False):
                    steps.append(half(st * 512, lo))
            if l >= 4:
                steps.append(lambda: nc.sync.dma_start(
                    out=din[l - 4][:, 0:KCOLS], in_=k_t[:, :]))
            return k_t, steps

        def gather(rnd):
            nc.gpsimd.collective_compute(
                "AllGather", mybir.AluOpType.bypass,
                replica_groups=GROUPS,
                ins=[din[rnd][:, :]], outs=[dout[rnd][:, :, :]],
            )

        def load_kv(u):
            rnd, r = (u - 4) // 2, (u - 4) % 2
            kg = kpool.tile([P, S], BF, tag="k_t")
            nc.sync.dma_start(out=kg[:, :], in_=dout[rnd][r, :, 0:KCOLS])
            return kg

        # ---- prologue: q0, k pair0, v group A (pairs 0,1) ----
        qh, kh = {}, {}
        qh[0], qsteps0 = make_q_steps(0)
        for s_ in qsteps0:
            s_()
        kh[0], ksteps = make_k_steps(0)
        for s_ in ksteps:
            s_()
        vA, vsteps = make_vq_steps(0)
        for s_ in vsteps:
            s_()
        vloc = {0: vA}

        # ---- main loop over positions ----
        kvg = {}
        for u in range(NHP):
            q_t = qh[u]
            v_q = vloc[u // 2]
            vh = u & 1
            k_t = kh[u] if u < 4 else kvg[u]

            for qt in range(SQ // 512):
                qsl = ts(qt, 512)
                ps_z0 = zp.tile([P, 512], F32, tag="z")
                ps_z1 = zp.tile([P, 512], F32, tag="z")
                for j2 in range(NJ // 2):
                    for i in range(2):
                        j = 2 * j2 + i
                        ps_s = sp.tile([P, 1024], F32, tag="s")
                        nc.tensor.matmul(ps_s[:, 0:512],
                                         k_t[0:64, ts(j, P)], q_t[0:64, qsl],
                                         start=True, stop=True)
                        nc.tensor.matmul(ps_s[:, 512:1024],
                                         k_t[64:P, ts(j, P)], q_t[64:P, qsl],
                                         start=True, stop=True)
                        e = epool.tile([P, 1024], BF, tag="e")
                        nc.scalar.activation(e[:, :], ps_s[:, :], EXP)
                        nc.tensor.matmul(ps_z0[0:65, :],
                                         v_q[:, j, vh, 0:65], e[:, 0:512],
                                         start=(j == 0), stop=(j == NJ - 1))
                        nc.tensor.matmul(ps_z1[0:65, :],
                                         v_q[:, j, vh, 65:130], e[:, 512:1024],
                                         start=(j == 0), stop=(j == NJ - 1))
                # normalize (lowest DVE priority except the last position,
                # whose zT writes gate the output-projection fillers)
                with tc.high_priority(offset=0 if u == NHP - 1 else NORM_PRI):
                    r0 = rbpool.tile([1, 512], F32, tag="rb", name="r0")
                    nc.vector.tensor_copy(r0[0:1, :], ps_z0[64:65, :])
                    rb0 = rbpool.tile([64, 512], F32, tag="rb")
                    nc.gpsimd.partition_broadcast(rb0[:, :], r0[0:1, :])
                    RECIP(rb0[:, :], rb0[:, :])
                    nc.vector.tensor_mul(zT[0:64, u, qsl], ps_z0[0:64, :],
                                         rb0[:, :])
                    r1 = rbpool.tile([1, 512], F32, tag="rb", name="r1")
                    nc.vector.tensor_copy(r1[0:1, :], ps_z1[64:65, :])
                    rb1 = rbpool.tile([64, 512], F32, tag="rb")
                    nc.gpsimd.partition_broadcast(rb1[:, :], r1[0:1, :])
                    RECIP(rb1[:, :], rb1[:, :])
                    nc.vector.tensor_mul(zT[64:P, u, qsl], ps_z1[0:64, :],
                                         rb1[:, :])

                if u == NHP - 1 and qt == 0:
                    with tc.high_priority(offset=FILLER):
                        for qc in range(4):
                            for dt in range(2):
                                outproj_step(qc, dt)

            # fillers for upcoming work. Exchange k pairs are computed and
            # staged early (rounds gather while early attention runs, with
            # positions 4-7 consuming them much later).
            with tc.high_priority(offset=FILLER):
                if u + 1 < NHP:
                    qh[u + 1], steps = make_q_steps(u + 1)
                    for s_ in steps:
                        s_()
                if u < 3:
                    # local k for pairs 1-3
                    kh[u + 1], steps = make_k_steps(u + 1)
                    for s_ in steps:
                        s_()
                if u == 0:
                    # exchange k pair (local slot 4, round 0, incl. stage)
                    _, steps = make_k_steps(4)
                    for s_ in steps:
                        s_()
                    gather(0)
                elif u == 1:
                    _, steps = make_k_steps(5)
                    for s_ in steps:
                        s_()
                    gather(1)
            if u % 2 == 1 and u // 2 + 1 < 4:
                with tc.high_priority(offset=FILLER_V):
                    vloc[u // 2 + 1], vsteps = make_vq_steps(u // 2 + 1)
                    for s_ in vsteps:
                        s_()
            # prefetch gathered k (emission after the gathers above, and
            # after local k3 so the k_t slot ring keeps k3 one position
            # ahead of its consumer)
            if u == 2:
                kvg[4] = load_kv(4)
                kvg[5] = load_kv(5)
            elif u == 3:
                kvg[6] = load_kv(6)
                kvg[7] = load_kv(7)

        # ---- output projection remainder ----
        for qc in range(4, SQ // P):
            for dt in range(2):
                outproj_step(qc, dt)


_NC_CACHE = {}


def get_nc(variant=None):
    if variant is None:
        variant = os.environ.get("KVARIANT", "")
    if variant not in _NC_CACHE:
        nc = bacc.Bacc("TRN2", target_bir_lowering=False, debug=False,
                       num_devices=N_CORES)
        with tile.TileContext(nc) as tc:
            build_kernel(nc, tc, variant)
        nc.compile()
        _NC_CACHE[variant] = nc
    return _NC_CACHE[variant]


def _bf(a):
    return np.ascontiguousarray(a.astype(ml_dtypes.bfloat16))


def shard_inputs(inputs):
    f = lambda n: np.asarray(inputs[n], dtype=np.float32)
    iq, ik, iv = f("input_query"), f("input_key"), f("input_value")
    wq, wk, wv = f("Wq"), f("Wk"), f("Wv")
    bq, bk, bv = f("bq"), f("bk"), f("bv")
    wo, bo = f("Wo"), f("bo")

    def pack_w(w):  # [h, D, DK] -> [128, 8, h*DK]
        h = w.shape[0]
        x = np.transpose(w, (1, 0, 2)).reshape(8, P, h * DK)
        return _bf(np.transpose(x, (1, 0, 2)))

    def pack_b(b):  # [h, DK] -> [128, h/2]
        x = b.reshape(b.shape[0] // 2, 2 * DK).T
        return np.ascontiguousarray(x)

    wq_p = pack_w(wq)
    wo_p = _bf(np.transpose(wo.reshape(8, P, 2, 512), (2, 1, 0, 3)))
    bq_p = _bf(pack_b(bq))

    in_maps = []
    for c in range(N_CORES):
        b_, half = c // 2, c % 2
        p = c % 2
        # k slots: pairs 0-3, then exchange pairs {4+p, 6+p}
        kpairs = [0, 1, 2, 3, 4 + p, 6 + p]
        kheads = [2 * q + e for q in kpairs for e in (0, 1)]
        wk_p = pack_w(wk[kheads])            # [P, 8, 768]
        wv_p = pack_w(wv)                    # [P, 8, 1024] (full, natural)
        bk_p = _bf(pack_b(bk[kheads]))       # [P, 6]
        bvb_p = np.broadcast_to(_bf(bv.reshape(1, H * DV)), (P, H * DV))

        xq_p = _bf(iq[b_, half * SQ:(half + 1) * SQ, :].T)
        xk_p = _bf(ik[b_].T)
        xv_p = _bf(iv[b_].T)
        blob = np.empty(BLOB_N, ml_dtypes.bfloat16)
        for off, arr in ((OFF_WQ, wq_p), (OFF_XQ, xq_p), (OFF_WK, wk_p),
                         (OFF_XK, xk_p), (OFF_WV, wv_p), (OFF_XV, xv_p),
                         (OFF_WO, wo_p), (OFF_BQ, bq_p), (OFF_BK, bk_p),
                         (OFF_BVB, bvb_p)):
            blob[off:off + arr.size] = arr.ravel()
        in_maps.append({"blob": blob})
    return in_maps


def kernel(**inputs):
    nc = get_nc()
    in_maps = shard_inputs(inputs)
    res = run_bass_kernel_spmd(nc, in_maps, core_ids=list(range(N_CORES)),
                               trace=False)
    out = np.empty((B, S, D), np.float32)
    for c in range(N_CORES):
        b_, half = c // 2, c % 2
        out[b_, half * SQ:(half + 1) * SQ, :] = \
            np.asarray(res.results[c]["out"]).astype(np.float32)
    # bo is added on the host (cheaper than a broadcast tile on device)
    out += np.asarray(inputs["bo"], dtype=np.float32)[None, None, :]
    return out


# revision 40
# speedup vs baseline: 1.0259x; 1.0259x over previous
"""MHA kernel for Trainium2, 8 NeuronCores.

Sharding: (batch, query-half) -> 8 shards. Core c handles batch c//2,
query rows (c%2)*1024:(c%2+1)*1024. Each core computes all 16 heads for its
1024 query rows; K/V projections for its batch are computed locally
(duplicated across the 2 cores sharing a batch). Output rows are disjoint
across cores -> no collectives.

Host-side marshaling (part of sharding): X slices are transposed and cast to
bf16 ([D, S] layout) so the device does zero transposes of X; weights are
pre-cast/pre-packed into the SBUF layouts the matmuls want (Wo as contiguous
column halves). ALL inputs are concatenated into ONE bf16 blob per core:
this environment charges ~30us of per-execution overhead per bound input
tensor, so 1 blob instead of 11 params saves ~300us. All loads alternate the
sync/vector HWDGE queues (scalar kept free for exp). Output is written bf16
(half the write bytes) and upcast to f32 on the host.

Per-core compute (bf16 matmuls, f32 PSUM):
  per head-pair hp: qT/kT [128 = pair-stacked 64+64, S] via 8-chunk
  accumulated matmuls over D; biases folded into the PSUM->SBUF copy as
  per-partition scalars. v natural [s, 130] (ones cols interleaved for
  softmax rowsums). scores^T per head via K=64 matmuls (head pair concurrent
  in PE row groups); E = exp(scores^T) on ScalarE; z'^T accumulated with the
  ones-column trick; z^T = z'^T[:64] * recip(z'^T[64]); out = z^T.T @ Wo+bo.

Scheduling: the Tile scheduler dispatches ready work per engine by priority
(emission order). Attention is emitted at natural priority; ALL projection
work for future head-pairs (q/k proj of hp+1, v proj of group g+1) is
emitted as LOW-priority fillers (tc.high_priority with negative offset).
The scheduler then slots projection matmuls into the PE-idle moments that
open up while ACT chews exp, keeping the PE dense so the HAM clock gate
stays at 8/8 (2.4 GHz) instead of oscillating to half clock.
"""

import os

import numpy as np
import ml_dtypes



import concourse.bass as bass
import concourse.tile as tile
from concourse import bacc, mybir
from concourse.bass import ds, ts
from concourse.bass_utils import run_bass_kernel_spmd

B, S, D = 4, 2048, 1024
H, DK, DV = 16, 64, 64
N_CORES = 8
SQ = S // 2  # query rows per core
P = 128
NHP = H // 2  # head pairs
F32 = mybir.dt.float32
BF = mybir.dt.bfloat16
EXP = mybir.ActivationFunctionType.Exp

# priority offsets for filler (projection) work: negative offset makes the
# scheduler treat these instructions as LATER than all attention work, so
# they only dispatch when the attention chain has nothing ready. q/k proj
# (which gates the next head-pair's scores, hence ACT's exp supply) gets a
# strictly better tier than vproj (which only gates zv, deps force it in
# time anyway).
FILLER = -10_000_000      # q/k proj steps
FILLER_V = -15_000_000    # v proj steps
NORM_PRI = -25_000_000    # normalize chain: lowest — its only consumer slack
                          # is zp recycling (one full qt of attention)


# blob layout (bf16 elements): every marshaled tensor concatenated in
# pipeline order. ~30us of per-exec runtime overhead PER bound input tensor
# makes a single input parameter worth ~300us vs 11 separate ones.
SZ_W = P * 8 * 1024          # wq/wk/wv/wo each
SZ_XQ = D * SQ
SZ_XKV = D * S
OFF_WQ = 0
OFF_XQ = OFF_WQ + SZ_W
OFF_WK = OFF_XQ + SZ_XQ
OFF_XK = OFF_WK + SZ_W
OFF_WV = OFF_XK + SZ_XKV
OFF_XV = OFF_WV + SZ_W
OFF_WO = OFF_XV + SZ_XKV
OFF_BQ = OFF_WO + SZ_W
OFF_BK = OFF_BQ + P * NHP
OFF_BVB = OFF_BK + P * NHP        # bv pre-broadcast to all 128 partitions
BLOB_N = OFF_BVB + P * H * DV


def build_kernel(nc, tc, VARIANT=""):
    blob = nc.declare_dram_parameter("blob", [BLOB_N], BF, isOutput=False).ap()
    # views into the blob, all bf16 (biases quantized to bf16 host-side)
    xqT_d = blob[ds(OFF_XQ, SZ_XQ)].rearrange("(p c) -> p c", p=D)
    xkT_d = blob[ds(OFF_XK, SZ_XKV)].rearrange("(p c) -> p c", p=D)
    xvT_d = blob[ds(OFF_XV, SZ_XKV)].rearrange("(p c) -> p c", p=D)
    wq_d = blob[ds(OFF_WQ, SZ_W)].rearrange("(p c k) -> p c k", p=P, c=8)
    wk_d = blob[ds(OFF_WK, SZ_W)].rearrange("(p c k) -> p c k", p=P, c=8)
    wv_d = blob[ds(OFF_WV, SZ_W)].rearrange("(p c k) -> p c k", p=P, c=8)
    wo_d = blob[ds(OFF_WO, SZ_W)].rearrange("(t p c k) -> t p c k", t=2, p=P, c=8)
    bq_d = blob[ds(OFF_BQ, P * NHP)].rearrange("(p c) -> p c", p=P)
    bk_d = blob[ds(OFF_BK, P * NHP)].rearrange("(p c) -> p c", p=P)
    bvb_d = blob[ds(OFF_BVB, P * H * DV)].rearrange("(p c) -> p c", p=P)
    # out in bf16: halves the write bytes; host upcasts to f32
    out = nc.declare_dram_parameter("out", [SQ, D], BF, isOutput=True).ap()

    import contextlib

    ctx = contextlib.ExitStack()
    with ctx:
        consts = ctx.enter_context(tc.tile_pool(name="consts", bufs=1))
        wpool = ctx.enter_context(tc.tile_pool(name="wpool", bufs=1))
        xtp = ctx.enter_context(tc.tile_pool(name="xtp", bufs=1))
        ztp = ctx.enter_context(tc.tile_pool(name="ztp", bufs=1))
        qkv = ctx.enter_context(tc.tile_pool(name="qkv", bufs=2))
        epool = ctx.enter_context(tc.tile_pool(name="epool", bufs=3))
        rpool = ctx.enter_context(tc.tile_pool(name="rpool", bufs=1))
        rbpool = ctx.enter_context(tc.tile_pool(name="rbpool", bufs=2))
        opool = ctx.enter_context(tc.tile_pool(name="opool", bufs=2))
        wopool = ctx.enter_context(tc.tile_pool(name="wopool", bufs=2))
        vqp = ctx.enter_context(tc.tile_pool(name="vqp", bufs=2))
        pp = ctx.enter_context(tc.tile_pool(name="pp", bufs=2, space=bass.MemorySpace.PSUM))
        sp = ctx.enter_context(tc.tile_pool(name="sp", bufs=2, space=bass.MemorySpace.PSUM))
        zp = ctx.enter_context(tc.tile_pool(name="zp", bufs=2, space=bass.MemorySpace.PSUM))

        # reciprocal_approx_fast requires base-partition-0 input (the
        # custom-DVE lowering drops the AP base partition); with the ones
        # column first, rowsums land on partition 0 and it's safe. ~5x
        # faster than exact reciprocal AND priced accurately by the
        # scheduler's cost model (exact recip is ~6x under-modeled, which
        # made the static schedule stall the PE stream at qt boundaries).
        RECIP = nc.vector.reciprocal_approx_fast
        # ---- constants (all on HWDGE queues — no SWDGE DMAs at all) ----
        bqc_bf = consts.tile([P, NHP], BF, tag="bqc_bf")
        bkc_bf = consts.tile([P, NHP], BF, tag="bkc_bf")
        nc.sync.dma_start(out=bqc_bf[:, :], in_=bq_d[:, :])
        nc.scalar.dma_start(out=bkc_bf[:, :], in_=bk_d[:, :])
        bqc = consts.tile([P, NHP], F32, tag="bqc")
        bkc = consts.tile([P, NHP], F32, tag="bkc")
        nc.vector.tensor_copy(bqc[:, :], bqc_bf[:, :])
        nc.vector.tensor_copy(bkc[:, :], bkc_bf[:, :])
        # pre-broadcast in the blob: straight loads, no partition_broadcast
        bvb = consts.tile([P, H * DV], BF, tag="bvb")
        bo_bc = consts.tile([P, D], BF, tag="bo_bc")

        wq_sb = wpool.tile([P, 8, H * DK], BF, tag="wq_sb")
        wk_sb = wpool.tile([P, 8, H * DK], BF, tag="wk_sb")
        wv_sb = wpool.tile([P, 8, H * DV], BF, tag="wv_sb")

        # ---- xT loads: [128, 8 dchunk, S] bf16 ----
        xqT = xtp.tile([P, 8, SQ], BF, tag="xqT")
        xkT = xtp.tile([P, 8, S], BF, tag="xkT")
        xvT = xtp.tile([P, 8, S], BF, tag="xvT")
        # all loads alternate sync/scalar in pipeline order (HWDGE dma_start
        # exists only on SP/ACT); no SWDGE (gpsimd) DMAs at all
        _k = [0]

        def E():
            e = (nc.sync, nc.scalar)[_k[0] % 2]
            _k[0] += 1
            return e

        # wq/xq interleaved per d-chunk: the first q-proj half-step (dc 0-3)
        # becomes ready after 2MB instead of 3MB of the stream
        for dc in range(8):
            E().dma_start(out=wq_sb[:, dc, :], in_=wq_d[:, dc, :])
            E().dma_start(out=xqT[:, dc, :], in_=xqT_d[ds(dc * P, P), :])
        E().dma_start(out=wk_sb[:, :, :], in_=wk_d[:, :, :])
        # xk in quarter-S pieces, s-major: the first k-proj block (st=0) is
        # ready after 1MB of xk instead of 4MB, so hp0's scores start ~8us
        # earlier
        for sq in range(4):
            for dc in range(8):
                E().dma_start(out=xkT[:, dc, ts(sq, 512)],
                              in_=xkT_d[ds(dc * P, P), ts(sq, 512)])
        E().dma_start(out=wv_sb[:, :, :], in_=wv_d[:, :, :])
        E().dma_start(out=bvb[:, :], in_=bvb_d[:, :])
        # xv in quarter-S pieces, s-major: vproj for the first s-chunks (and
        # hence hp0's zv) can start after 1/4 of the xv bytes instead of all
        for sq in range(4):
            for dc in range(8):
                E().dma_start(out=xvT[:, dc, ts(sq, 512)],
                              in_=xvT_d[ds(dc * P, P), ts(sq, 512)])

        # wo halves pre-packed contiguous host-side: wo_d[dt] = [P, 8, 512]
        wo_sb0 = wopool.tile([P, 8, 512], BF, tag="wo_sb")
        E().dma_start(out=wo_sb0[:, :, :], in_=wo_d[0, :, :, :])
        wo_sb1 = wopool.tile([P, 8, 512], BF, tag="wo_sb")
        E().dma_start(out=wo_sb1[:, :, :], in_=wo_d[1, :, :, :])

        # z^T accumulator: [128 = dv(h0)|dv(h1), 8 head-pairs, 1024 q]
        zT = ztp.tile([P, NHP, SQ], BF, tag="zT")

        NJ = S // P  # 16 k-chunks

        def outproj_step(qc, dt):
            wo_sb = wo_sb0 if dt == 0 else wo_sb1
            ps = pp.tile([P, 512], F32, tag="proj")
            for fc in range(8):
                nc.tensor.matmul(ps[:, :], zT[:, fc, ts(qc, P)],
                                 wo_sb[:, fc, :],
                                 start=(fc == 0), stop=(fc == 7))
            o_t = opool.tile([P, 512], BF, tag="o")
            nc.vector.tensor_copy(o_t[:, :], ps[:, :])
            nc.sync.dma_start(out=out[ts(qc, P), ts(dt, 512)], in_=o_t[:, :])

        # ---- v projection steps for a 2-head-pair group ----
        def make_vq_steps(g2):
            # v natural for 2 head-pairs: [128 s, 16 sc, 2 hp, 130]
            v_q = vqp.tile([P, S // P, 2, 130], BF, tag="v_q")
            vv = v_q.rearrange("p s h (a c) -> p s h a c", a=2)
            gsl = ds(g2 * 256, 256)  # 2 hp = 256 cols in (h k) layout
            steps = [lambda: nc.vector.memset(vv[:, :, :, :, 64:65], 1.0)]

            def step(sc):
                def emit():
                    ps = pp.tile([P, 512], F32, tag="proj")
                    for dc in range(8):
                        nc.tensor.matmul(ps[:, 0:256], xvT[:, dc, ts(sc, P)],
                                         wv_sb[:, dc, gsl],
                                         start=(dc == 0), stop=(dc == 7))
                    nc.vector.tensor_add(
                        vv[:, sc, :, :, 0:64],
                        ps[:, 0:256].rearrange("p (h a c) -> p h a c", h=2, a=2),
                        bvb[:, gsl].rearrange("p (h a c) -> p h a c", h=2, a=2),
                    )
                return emit

            steps += [step(sc) for sc in range(S // P)]
            return v_q, steps

        # ---- q/k projection steps for one head-pair ----
        def make_proj_steps(hp):
            hsl = ts(hp, P)
            q_t = qkv.tile([P, SQ], BF, tag="q_t")
            k_t = qkv.tile([P, S], BF, tag="k_t")
            steps = []
            state = {}

            def half(w_sb, x_T, dst, bias, col, lo):
                def emit():
                    if lo:
                        ps = pp.tile([P, 512], F32, tag="proj")
                        state[(dst.name, col)] = ps
                    else:
                        ps = state.pop((dst.name, col))
                    for dc in range(4):
                        d = dc if lo else dc + 4
                        nc.tensor.matmul(ps[:, :], w_sb[:, d, hsl],
                                         x_T[:, d, ds(col, 512)],
                                         start=(d == 0), stop=(d == 7))
                    if not lo:
                        nc.vector.tensor_scalar_add(
                            dst[:, ds(col, 512)], ps[:, :], bias)
                return emit

            # q qt0 first, then k st0-3, then q qt1: the attention needs
            # (q qt0, k st0) to START scores; k st1-3 feed the j2 loop
            # progressively; q qt1 isn't needed until the second half
            for lo in (True, False):
                steps.append(half(wq_sb, xqT, q_t, bqc[:, hp:hp + 1], 0, lo))
            for st in range(S // 512):
                for lo in (True, False):
                    steps.append(half(wk_sb, xkT, k_t,
                                      bkc[:, hp:hp + 1], st * 512, lo))
            for lo in (True, False):
                steps.append(half(wq_sb, xqT, q_t, bqc[:, hp:hp + 1], 512, lo))
            return q_t, k_t, steps

        # ---- prologue: hp0 projections + vproj g0 at natural priority ----
        qh, kh, vqh = {}, {}, {}
        qh[0], kh[0], steps0 = make_proj_steps(0)
        for s_ in steps0:
            s_()
        vqh[0], vsteps0 = make_vq_steps(0)
        for s_ in vsteps0:
            s_()

        # ---- main loop over head-pairs ----
        for hp in range(NHP):
            q_t, k_t, v_q = qh[hp], kh[hp], vqh[hp // 2]

            for qt in range(SQ // 512):
                qsl = ts(qt, 512)
                ps_z0 = zp.tile([P, 512], F32, tag="z")
                ps_z1 = zp.tile([P, 512], F32, tag="z")
                for j2 in range(NJ // 2):
                    # scores psum packed per k-chunk: tile = [h0 | h1] for
                    # ONE chunk. Both heads' matmuls share one allocation,
                    # become ready together, and dispatch back-to-back —
                    # the PE then runs them CONCURRENTLY in disjoint row
                    # groups (K=64 each), ~2x on the scores phase.
                    for i in range(2):
                        j = 2 * j2 + i
                        ps_s = sp.tile([P, 1024], F32, tag="s")
                        nc.tensor.matmul(ps_s[:, 0:512],
                                         k_t[0:64, ts(j, P)], q_t[0:64, qsl],
                                         start=True, stop=True)
                        nc.tensor.matmul(ps_s[:, 512:1024],
                                         k_t[64:P, ts(j, P)], q_t[64:P, qsl],
                                         start=True, stop=True)
                        e = epool.tile([P, 1024], BF, tag="e")
                        nc.scalar.activation(e[:, :], ps_s[:, :], EXP)
                        nc.tensor.matmul(ps_z0[0:65, :],
                                         v_q[:, j, hp & 1, 0:65],
                                         e[:, 0:512],
                                         start=(j == 0), stop=(j == NJ - 1))
                        nc.tensor.matmul(ps_z1[0:65, :],
                                         v_q[:, j, hp & 1, 65:130],
                                         e[:, 512:1024],
                                         start=(j == 0), stop=(j == NJ - 1))
                # normalize: z = z' * recip(rowsum)
                # normalize at the lowest DVE priority: if it outranked
                # proj/vproj psum drains on DVE, pp slots would fill and
                # stall the PE. Chain per head: DVE copy of the raw rowsum
                # row to SBUF (frees the psum bank fast), gpsimd broadcast,
                # approx-reciprocal in place on [64,512] (elementwise, base
                # 0 — safe for the custom op), then the multiply. All ops
                # here are ~1 cyc/elem and priced correctly by the cost
                # model, unlike exact reciprocal (~6x under-modeled, which
                # made the static schedule stall the PE stream).
                # EXCEPTION: the last head-pair's normalize gates the
                # output-projection fillers (zT writes) — run it at natural
                # priority so the outproj can hide under qt1's attention.
                with tc.high_priority(offset=0 if hp == NHP - 1 else NORM_PRI):
                    r0 = rpool.tile([1, 512], F32, tag="r")
                    nc.vector.tensor_copy(r0[0:1, :], ps_z0[64:65, :])
                    rb0 = rbpool.tile([64, 512], F32, tag="rb")
                    nc.gpsimd.partition_broadcast(rb0[:, :], r0[0:1, :])
                    RECIP(rb0[:, :], rb0[:, :])
                    nc.vector.tensor_mul(zT[0:64, hp, qsl], ps_z0[0:64, :],
                                         rb0[:, :])
                    r1 = rpool.tile([1, 512], F32, tag="r")
                    nc.vector.tensor_copy(r1[0:1, :], ps_z1[64:65, :])
                    rb1 = rbpool.tile([64, 512], F32, tag="rb")
                    nc.gpsimd.partition_broadcast(rb1[:, :], r1[0:1, :])
                    RECIP(rb1[:, :], rb1[:, :])
                    nc.vector.tensor_mul(zT[64:P, hp, qsl], ps_z1[0:64, :],
                                         rb1[:, :])

                # qc 0-3 outproj depends only on qt0 rows of zT: as fillers
                # during the last head-pair's qt1 attention
                if hp == NHP - 1 and qt == 0:
                    with tc.high_priority(offset=FILLER):
                        for qc in range(4):
                            for dt in range(2):
                                outproj_step(qc, dt)

            # fillers for upcoming work, at filler priority: the scheduler
            # slots these into PE-idle moments of the attention above
            # emission order also sets the psum pool (pp) slot-ring order:
            # proj(hp+1) tiles before vproj tiles, and vproj(g+1) emitted
            # after the ODD hp so its ring position matches when it's needed
            # (during hp+1 = even attention, ahead of proj(hp+2)).
            with tc.high_priority(offset=FILLER):
                if hp + 1 < NHP:
                    qh[hp + 1], kh[hp + 1], steps = make_proj_steps(hp + 1)
                    for s_ in steps:
                        s_()
            if hp % 2 == 1 and hp // 2 + 1 < NHP // 2:
                with tc.high_priority(offset=FILLER_V):
                    vqh[hp // 2 + 1], vsteps = make_vq_steps(hp // 2 + 1)
                    for s_ in vsteps:
                        s_()

        # ---- output projection remainder (qc 4-7 need qt1 rows of zT) ----
        for qc in range(4, SQ // P):
            for dt in range(2):
                outproj_step(qc, dt)


_NC_CACHE = {}


def get_nc(variant=None):
    if variant is None:
        variant = os.environ.get("KVARIANT", "")
    if variant not in _NC_CACHE:
        nc = bacc.Bacc("TRN2", target_bir_lowering=False, debug=False,
                       num_devices=N_CORES)
        with tile.TileContext(nc) as tc:
            build_kernel(nc, tc, variant)
        nc.compile()
        _NC_CACHE[variant] = nc
    return _NC_CACHE[variant]


def _bf(a):
    return np.ascontiguousarray(a.astype(ml_dtypes.bfloat16))


def shard_inputs(inputs):
    f = lambda n: np.asarray(inputs[n], dtype=np.float32)
    iq, ik, iv = f("input_query"), f("input_key"), f("input_value")
    wq, wk, wv = f("Wq"), f("Wk"), f("Wv")
    bq, bk, bv = f("bq"), f("bk"), f("bv")
    wo, bo = f("Wo"), f("bo")

    # weights -> [128 d-in-chunk, 8 dchunk, (h k)] bf16
    def pack_w(w):  # [H, D, DK]
        x = np.transpose(w, (1, 0, 2)).reshape(8, P, H * DK)  # [dc, dp, (h k)]
        return _bf(np.transpose(x, (1, 0, 2)))  # [128, 8, H*DK]

    # biases -> [128 pair-stacked, 8 hp] f32
    def pack_b(b):  # [H, DK]
        x = b.reshape(NHP, 2 * DK).T  # [128, NHP]
        return np.ascontiguousarray(x)

    wq_p, wk_p, wv_p = pack_w(wq), pack_w(wk), pack_w(wv)  # [P, 8, 1024]
    wo_p = _bf(np.transpose(wo.reshape(8, P, 2, 512), (2, 1, 0, 3)))  # [2,P,8,512]
    bq_p, bk_p = _bf(pack_b(bq)), _bf(pack_b(bk))  # [P, 8]
    bvb_p = np.broadcast_to(_bf(bv.reshape(1, H * DV)), (P, H * DV))

    in_maps = []
    for c in range(N_CORES):
        b_, half = c // 2, c % 2
        xq_p = _bf(iq[b_, half * SQ:(half + 1) * SQ, :].T)
        xk_p = _bf(ik[b_].T)
        xv_p = _bf(iv[b_].T)
        blob = np.empty(BLOB_N, ml_dtypes.bfloat16)
        for off, arr in ((OFF_WQ, wq_p), (OFF_XQ, xq_p), (OFF_WK, wk_p),
                         (OFF_XK, xk_p), (OFF_WV, wv_p), (OFF_XV, xv_p),
                         (OFF_WO, wo_p), (OFF_BQ, bq_p), (OFF_BK, bk_p),
                         (OFF_BVB, bvb_p)):
            blob[off:off + arr.size] = arr.ravel()
        in_maps.append({"blob": blob})
    return in_maps


def kernel(**inputs):
    nc = get_nc()
    in_maps = shard_inputs(inputs)
    res = run_bass_kernel_spmd(nc, in_maps, core_ids=list(range(N_CORES)),
                               trace=False)
    out = np.empty((B, S, D), np.float32)
    for c in range(N_CORES):
        b_, half = c // 2, c % 2
        out[b_, half * SQ:(half + 1) * SQ, :] = \
            np.asarray(res.results[c]["out"]).astype(np.float32)
    # bo added on the host: saves a [128, D] broadcast tile + its load
    out += np.asarray(inputs["bo"], dtype=np.float32)[None, None, :]
    return out
